# revision 6
# baseline (speedup 1.0000x reference)
"""MoE layer (SwiGLU experts, top-2 routing) on 8 Trainium2 NeuronCores.

Strategy (expert parallelism, per the sharding hint):
  - The router (a [N,8] matmul + softmax + top-2, ~0.01% of total FLOPs) is
    computed host-side in float64; it determines the token->expert dispatch.
  - Token dispatch/combine (the "all-to-all") is done host-side: each core e
    receives expert e's weights plus the tokens routed to expert e, padded to
    a uniform capacity C (multiple of 128, same on all cores for SPMD).
  - Each core runs the heavy compute in bf16 (full PE rate, rel-err ~4e-3
    which is well under the 2e-2 budget); accumulation stays fp32 in PSUM.
  - The per-token combine weight is folded into a second, pre-scaled copy of
    the token slab that feeds the up-projection: silu(x@wg) * ((s*x)@wu)
    equals s * (silu(x@wg) * (x@wu)), so stage 2 needs no scaling at all.
  - Weights are host-permuted into tile-contiguous layouts so every weight
    DMA reads one contiguous 2-4KB chunk per partition (descriptor-cheap).
  - Host scatter-adds each expert's (transposed) output back into the output.

Device kernel structure (per core):
  Tokens are processed in groups (a small 128-token first group hides the
  initial token-slab DMA behind real work; then ~1024-token groups with
  512-wide matmul chunks). Stage 1 computes hT[f, c] = silu(wg.T x) *
  (wu.T xs) for all F=4096 rows of the group, accumulating over D=1024 in
  PSUM, gate/up banks drained by ScalarE (silu) and VectorE (mul, cast to
  bf16) into SBUF. Stage 2 is transposed: stationary = wd tile [128f, 128d],
  moving = hT[f, c-span], accumulating yT[d, c] over F in PSUM (32 steps),
  so wd streams exactly once per group. Output is written transposed
  (yT [D, C]); the host transposes during the combine.
"""

import os
import sys

sys.path.insert(0, "/opt/trn_rl_repo")
import numpy as np

P = 128
D_MODEL = 1024
D_FF = 4096
N_EXPERTS = 8
TOP_K = 2
FTB = 16  # f-tiles per wd DMA (4KB per partition, contiguous)

LAST_EXEC_NS = None
_programs = {}


def _ensure_axon_hooks():
    """The agent image's antenv lacks axon_hooks; reconstruct it so
    trace=True works (NTFF profiling via libaxon_pjrt ctypes hook)."""
    import types

    try:
        import antenv.axon_hooks  # noqa: F401

        return
    except ImportError:
        pass
    try:
        import antenv

        mod = types.ModuleType("antenv.axon_hooks")
        _hook = [None]
        mod.set_axon_ntff_profile_hook = lambda h: _hook.__setitem__(0, h)
        mod.get_axon_ntff_profile_hook = lambda: _hook[0]
        sys.modules["antenv.axon_hooks"] = mod
        antenv.axon_hooks = mod
        if "/root/.axon_site" not in sys.path:
            sys.path.insert(0, "/root/.axon_site")
        from trn_agent_boot.trn_boot import _ntff_profile_via_ctypes

        mod.set_axon_ntff_profile_hook(
            _ntff_profile_via_ctypes("/opt/axon/libaxon_pjrt.so")
        )
        import concourse.bass_utils as bu

        bu.upload_artifacts = lambda tmpdir: f"local://{tmpdir}"
    except Exception:
        pass


def _group_plan(C):
    """A small 128-token first group (to hide x-slab priming behind real
    matmuls), then ~1024-token groups; short remainders fold into the last
    group so chunks stay wide."""
    if C <= 512:
        return [C]
    rest = C - 128
    k, r = divmod(rest, 1024)
    if r == 0:
        sizes = [1024] * k
    elif r >= 512 or k == 0:
        sizes = [1024] * k + [r]
    else:
        sizes = [1024] * (k - 1) + [1024 + r]
    return [128] + sizes


def _chunk_plan(gc):
    """512-wide chunks with one (possibly narrow) tail chunk."""
    spans = []
    c0 = 0
    while c0 < gc:
        w = min(512, gc - c0)
        spans.append((c0, w))
        c0 += w
    return spans


def _build_program(C):
    import concourse.bacc as bacc
    import concourse.mybir as mybir
    from concourse.tile import TileContext

    fp32 = mybir.dt.float32
    bf16 = mybir.dt.bfloat16
    D, F = D_MODEL, D_FF
    DT, FT = D // P, F // P
    DB = D // P
    silu_fn = mybir.ActivationFunctionType.Silu
    mult_op = mybir.AluOpType.mult

    nc = bacc.Bacc(
        "TRN2", target_bir_lowering=False, debug=False, num_devices=N_EXPERTS
    )
    xT = nc.dram_tensor("xT", [D, C], bf16, kind="ExternalInput")
    xuT = nc.dram_tensor("xuT", [D, C], bf16, kind="ExternalInput")
    # host-permuted tile-contiguous layouts:
    #   wgx/wux [128p, 32ft, 8dt, 128f'] ; wdx [128p, 8db, FT/FTB, FTB, 128d']
    wgx = nc.dram_tensor("wgx", [P, FT * DT * P], bf16, kind="ExternalInput")
    wux = nc.dram_tensor("wux", [P, FT * DT * P], bf16, kind="ExternalInput")
    wdx = nc.dram_tensor("wdx", [P, DB * FT * P], bf16, kind="ExternalInput")
    yT = nc.dram_tensor("yT", [D, C], fp32, kind="ExternalOutput")

    xT_r = xT.ap().rearrange("(dt p) c -> p dt c", p=P)
    xuT_r = xuT.ap().rearrange("(dt p) c -> p dt c", p=P)
    wg_r = wgx.ap().rearrange("p (ft dt f) -> p ft dt f", ft=FT, dt=DT)
    wu_r = wux.ap().rearrange("p (ft dt f) -> p ft dt f", ft=FT, dt=DT)
    wd_r = wdx.ap().rearrange(
        "p (db fb fi d) -> p db fb fi d", db=DB, fb=FT // FTB, fi=FTB
    )
    yT_ap = yT.ap()

    sizes = _group_plan(C)
    groups = []
    g0 = 0
    for gc in sizes:
        groups.append((g0, gc))
        g0 += gc
    gmax = max(sizes)

    with TileContext(nc) as tc:
        with (
            tc.tile_pool(name="warm", bufs=1) as warm_pool,
            tc.tile_pool(name="xg", bufs=2) as xg_pool,
            tc.tile_pool(name="xu", bufs=2) as xu_pool,
            tc.tile_pool(name="wgu", bufs=4) as wgu_pool,
            tc.tile_pool(name="ht", bufs=FT + 4) as ht_pool,
            tc.tile_pool(name="wdp", bufs=4) as wd_pool,
            tc.tile_pool(name="act", bufs=2) as act_pool,
            tc.tile_pool(name="out", bufs=4) as out_pool,
            tc.tile_pool(name="ps1", bufs=1, space="PSUM") as ps1_pool,
            tc.tile_pool(name="ps2", bufs=6, space="PSUM") as ps2_pool,
        ):
            # Warm-up: keep TensorE busy while the first tiles stream in, so
            # the HAM clock gate reaches 2.4 GHz before real matmuls start.
            wsrc = warm_pool.tile([P, 512], bf16, name="wsrc")
            nc.vector.memset(wsrc.bitcast(fp32)[:], 0.0)
            wps = ps1_pool.tile([P, 512], fp32, name="psg")
            for wi in range(60):
                nc.tensor.matmul(
                    wps[:, :256],
                    wsrc[:, :P],
                    wsrc[:, :256],
                    start=(wi == 0),
                    stop=(wi == 59),
                )
            for g0, gc in groups:
                spans = _chunk_plan(gc)

                # hoist the first f-tile's weights ahead of the token slab so
                # stage 1 can start as soon as xg's first slice lands
                wgu_pre = {}
                for ft in range(1):
                    wgt = wgu_pool.tile([P, DT, P], bf16, name="wgt")
                    nc.sync.dma_start(out=wgt[:], in_=wg_r[:, ft, :, :])
                    wut = wgu_pool.tile([P, DT, P], bf16, name="wut")
                    nc.sync.dma_start(out=wut[:], in_=wu_r[:, ft, :, :])
                    wgu_pre[ft] = (wgt, wut)

                xg = xg_pool.tile([P, DT, gmax], bf16, name="xg")
                xu = xu_pool.tile([P, DT, gmax], bf16, name="xu")
                # per-dt loads (gate slab first) so stage-1 matmuls can start
                # on the first slice
                for dt_i in range(DT):
                    nc.sync.dma_start(
                        out=xg[:, dt_i, :gc], in_=xT_r[:, dt_i, g0 : g0 + gc]
                    )
                for dt_i in range(DT):
                    nc.sync.dma_start(
                        out=xu[:, dt_i, :gc], in_=xuT_r[:, dt_i, g0 : g0 + gc]
                    )

                # ---- stage 1: hT[f, c] = silu(wg.T x) * (wu.T xs) ----
                ht_tiles = []
                for ft in range(FT):
                    if ft in wgu_pre:
                        wgt, wut = wgu_pre.pop(ft)
                    else:
                        wgt = wgu_pool.tile([P, DT, P], bf16, name="wgt")
                        nc.sync.dma_start(out=wgt[:], in_=wg_r[:, ft, :, :])
                        wut = wgu_pool.tile([P, DT, P], bf16, name="wut")
                        nc.sync.dma_start(out=wut[:], in_=wu_r[:, ft, :, :])
                    ht = ht_pool.tile([P, gmax], bf16, name="ht")
                    ht_tiles.append(ht)
                    for c0, cw in spans:
                        psg = ps1_pool.tile([P, 512], fp32, name="psg")
                        for dt_i in range(DT):
                            nc.tensor.matmul(
                                psg[:, :cw],
                                wgt[:, dt_i, :],
                                xg[:, dt_i, c0 : c0 + cw],
                                start=(dt_i == 0),
                                stop=(dt_i == DT - 1),
                            )
                        psu = ps1_pool.tile([P, 512], fp32, name="psu")
                        for dt_i in range(DT):
                            nc.tensor.matmul(
                                psu[:, :cw],
                                wut[:, dt_i, :],
                                xu[:, dt_i, c0 : c0 + cw],
                                start=(dt_i == 0),
                                stop=(dt_i == DT - 1),
                            )
                        sil = act_pool.tile([P, 512], fp32, name="sil")
                        nc.scalar.activation(sil[:, :cw], psg[:, :cw], silu_fn)
                        nc.vector.tensor_tensor(
                            out=ht[:, c0 : c0 + cw],
                            in0=sil[:, :cw],
                            in1=psu[:, :cw],
                            op=mult_op,
                        )

                # ---- stage 2 (transposed): yT[d, c] = sum_f wd[f, d] hT[f, c]
                # stationary = wd tile [128f, 128d], moving = hT span; wd
                # streams exactly once per group.
                for db in range(DB):
                    ps_out = [
                        ps2_pool.tile([P, 512], fp32, name="pso") for _ in spans
                    ]
                    for fb in range(FT // FTB):
                        wdt = wd_pool.tile([P, FTB, P], bf16, name="wdt")
                        nc.scalar.dma_start(
                            out=wdt[:], in_=wd_r[:, db, fb, :, :]
                        )
                        for fi in range(FTB):
                            ft = fb * FTB + fi
                            for si, (c0, cw) in enumerate(spans):
                                nc.tensor.matmul(
                                    ps_out[si][:, :cw],
                                    wdt[:, fi, :],
                                    ht_tiles[ft][:, c0 : c0 + cw],
                                    start=(ft == 0),
                                    stop=(ft == FT - 1),
                                )
                    for si, (c0, cw) in enumerate(spans):
                        ot = out_pool.tile([P, 512], fp32, name="ot")
                        if si % 2 == 0:
                            nc.vector.tensor_scalar_mul(
                                ot[:, :cw], ps_out[si][:, :cw], 1.0
                            )
                        else:
                            # spread evictions across engines so the bank
                            # ring frees faster at d-tile boundaries
                            nc.scalar.activation(
                                ot[:, :cw],
                                ps_out[si][:, :cw],
                                mybir.ActivationFunctionType.Copy,
                            )
                        # gpsimd/scalar queues: the sync queue must stay clear
                        # for the next group's token-slab prefetch
                        dma_eng = nc.gpsimd if si % 2 == 0 else nc.scalar
                        dma_eng.dma_start(
                            out=yT_ap[db * P : (db + 1) * P, g0 + c0 : g0 + c0 + cw],
                            in_=ot[:, :cw],
                        )
    nc.compile()
    return nc


def _get_program(C):
    if C not in _programs:
        _programs[C] = _build_program(C)
    return _programs[C]


def _route(xf, router_w):
    """Host router, float64 (all f32 evaluation orders agree on this input's
    top-2 sets; f64 is the stable reference ranking). Mirrors
    softmax -> top_k(2) -> renormalize from the reference."""
    logits = xf.astype(np.float64) @ router_w.astype(np.float64).T
    logits -= logits.max(axis=-1, keepdims=True)
    sm = np.exp(logits)
    sm /= sm.sum(axis=-1, keepdims=True)
    top = np.argsort(-sm, axis=-1, kind="stable")[:, :TOP_K]
    tsc = np.take_along_axis(sm, top, axis=1)
    tsc = tsc / tsc.sum(axis=-1, keepdims=True)
    return top, tsc


def _permute_wgu(w, bf):
    """[D, F] -> tile-contiguous [128p, (32ft 8dt 128f')]"""
    D, F = w.shape
    v = w.reshape(D // P, P, F // P, P).transpose(1, 2, 0, 3)
    return np.ascontiguousarray(v.astype(bf).reshape(P, -1))


def _permute_wd(w, bf):
    """[F, D] -> tile-contiguous [128p, (8db FT/FTB FTB 128d')]"""
    F, D = w.shape
    nfb = F // P // FTB
    v = w.reshape(nfb, FTB, P, D // P, P).transpose(2, 3, 0, 1, 4)
    return np.ascontiguousarray(v.astype(bf).reshape(P, -1))


def kernel(x, router_w, w_gate, w_up, w_down):
    global LAST_EXEC_NS
    import ml_dtypes
    from concourse.bass_utils import run_bass_kernel_spmd

    bf = ml_dtypes.bfloat16

    trace = os.environ.get("MOE_TRACE", "0") == "1"
    if trace:
        _ensure_axon_hooks()

    x = np.asarray(x, dtype=np.float32)
    router_w = np.asarray(router_w, dtype=np.float32)

    B, T, D = x.shape
    N = B * T
    xf = np.ascontiguousarray(x.reshape(N, D))

    top, tsc = _route(xf, router_w)

    tok_rows = []
    tok_wts = []
    for e in range(N_EXPERTS):
        mask = top == e
        rows = np.nonzero(mask.any(axis=1))[0]
        wts = tsc[mask].astype(np.float32)
        tok_rows.append(rows)
        tok_wts.append(wts)

    cmax = max(max(len(r) for r in tok_rows), 1)
    C = max(((cmax + P - 1) // P) * P, 256)

    nc = _get_program(C)

    in_maps = []
    for e in range(N_EXPERTS):
        rows = tok_rows[e]
        xg = np.zeros((C, D), np.float32)
        xg[: len(rows)] = xf[rows]
        xs = np.zeros((C, D), np.float32)
        xs[: len(rows)] = xf[rows] * tok_wts[e][:, None]
        in_maps.append(
            {
                "xT": np.ascontiguousarray(xg.T.astype(bf)),
                "xuT": np.ascontiguousarray(xs.T.astype(bf)),
                "wgx": _permute_wgu(np.asarray(w_gate[e], np.float32), bf),
                "wux": _permute_wgu(np.asarray(w_up[e], np.float32), bf),
                "wdx": _permute_wd(np.asarray(w_down[e], np.float32), bf),
            }
        )

    res = run_bass_kernel_spmd(nc, in_maps, list(range(N_EXPERTS)), trace=trace)
    if trace:
        LAST_EXEC_NS = res.exec_time_ns

    out = np.zeros((N, D), np.float32)
    for e in range(N_EXPERTS):
        rows = tok_rows[e]
        out[rows] += res.results[e]["yT"][:, : len(rows)].T
    return out.reshape(B, T, D)


# revision 10
# speedup vs baseline: 1.0603x; 1.0603x over previous
"""MoE layer (SwiGLU experts, top-2 routing) on 8 Trainium2 NeuronCores.

Strategy (expert parallelism, per the sharding hint):
  - The router (a [N,8] matmul + softmax + top-2, ~0.01% of total FLOPs) is
    computed host-side in float64; it determines the token->expert dispatch.
  - Token dispatch/combine (the "all-to-all") is done host-side: each core e
    receives expert e's weights plus the tokens routed to expert e, padded to
    a uniform capacity C (multiple of 128, same on all cores for SPMD).
  - Each core runs the heavy compute in bf16 (full PE rate, rel-err ~4e-3
    which is well under the 2e-2 budget); accumulation stays fp32 in PSUM.
  - The per-token combine weight is folded into a second, pre-scaled copy of
    the token slab that feeds the up-projection: silu(x@wg) * ((s*x)@wu)
    equals s * (silu(x@wg) * (x@wu)), so stage 2 needs no scaling at all.
  - Weights are host-permuted into tile-contiguous layouts so every weight
    DMA reads one contiguous 2-4KB chunk per partition (descriptor-cheap).
  - Host scatter-adds each expert's (transposed) output back into the output.

Device kernel structure (per core):
  Tokens are processed in groups (a small 128-token first group hides the
  initial token-slab DMA behind real work; then ~1024-token groups with
  512-wide matmul chunks). Stage 1 computes hT[f, c] = silu(wg.T x) *
  (wu.T xs) for all F=4096 rows of the group, accumulating over D=1024 in
  PSUM, gate/up banks drained by ScalarE (silu) and VectorE (mul, cast to
  bf16) into SBUF. Stage 2 is transposed: stationary = wd tile [128f, 128d],
  moving = hT[f, c-span], accumulating yT[d, c] over F in PSUM (32 steps),
  so wd streams exactly once per group. Output is written transposed
  (yT [D, C]); the host transposes during the combine.
"""

import os
import sys

sys.path.insert(0, "/opt/trn_rl_repo")
import numpy as np

P = 128
D_MODEL = 1024
D_FF = 4096
N_EXPERTS = 8
TOP_K = 2
FTB = 16  # f-tiles per wd DMA (4KB per partition, contiguous)

LAST_EXEC_NS = None
_programs = {}


def _ensure_axon_hooks():
    """The agent image's antenv lacks axon_hooks; reconstruct it so
    trace=True works (NTFF profiling via libaxon_pjrt ctypes hook)."""
    import types

    try:
        import antenv.axon_hooks  # noqa: F401

        return
    except ImportError:
        pass
    try:
        import antenv

        mod = types.ModuleType("antenv.axon_hooks")
        _hook = [None]
        mod.set_axon_ntff_profile_hook = lambda h: _hook.__setitem__(0, h)
        mod.get_axon_ntff_profile_hook = lambda: _hook[0]
        sys.modules["antenv.axon_hooks"] = mod
        antenv.axon_hooks = mod
        if "/root/.axon_site" not in sys.path:
            sys.path.insert(0, "/root/.axon_site")
        from trn_agent_boot.trn_boot import _ntff_profile_via_ctypes

        mod.set_axon_ntff_profile_hook(
            _ntff_profile_via_ctypes("/opt/axon/libaxon_pjrt.so")
        )
        import concourse.bass_utils as bu

        bu.upload_artifacts = lambda tmpdir: f"local://{tmpdir}"
    except Exception:
        pass


def _group_plan(C):
    """~1024-token groups (512-wide chunks); short remainders fold into the
    last group so chunks stay wide. Groups must be large: each group streams
    the full wg/wu (16.8MB), so a small group is weight-DMA-bound."""
    k, r = divmod(C, 1024)
    if k == 0:
        return [C]
    if r == 0:
        return [1024] * k
    if r >= 512:
        return [1024] * k + [r]
    return [1024] * (k - 1) + [1024 + r]


def _chunk_plan(gc):
    """512-wide chunks with one (possibly narrow) tail chunk."""
    spans = []
    c0 = 0
    while c0 < gc:
        w = min(512, gc - c0)
        spans.append((c0, w))
        c0 += w
    return spans


def _build_program(C):
    import concourse.bacc as bacc
    import concourse.mybir as mybir
    from concourse.tile import TileContext

    fp32 = mybir.dt.float32
    bf16 = mybir.dt.bfloat16
    D, F = D_MODEL, D_FF
    DT, FT = D // P, F // P
    DB = D // P
    silu_fn = mybir.ActivationFunctionType.Silu
    mult_op = mybir.AluOpType.mult

    nc = bacc.Bacc(
        "TRN2", target_bir_lowering=False, debug=False, num_devices=N_EXPERTS
    )
    xT = nc.dram_tensor("xT", [D, C], bf16, kind="ExternalInput")
    xuT = nc.dram_tensor("xuT", [D, C], bf16, kind="ExternalInput")
    # host-permuted tile-contiguous layouts:
    #   wgx/wux [128p, 32ft, 8dt, 128f'] ; wdx [128p, 8db, FT/FTB, FTB, 128d']
    wgx = nc.dram_tensor("wgx", [P, FT * DT * P], bf16, kind="ExternalInput")
    wux = nc.dram_tensor("wux", [P, FT * DT * P], bf16, kind="ExternalInput")
    wdx = nc.dram_tensor("wdx", [P, DB * FT * P], bf16, kind="ExternalInput")
    yT = nc.dram_tensor("yT", [D, C], fp32, kind="ExternalOutput")

    xT_r = xT.ap().rearrange("(dt p) c -> p dt c", p=P)
    xuT_r = xuT.ap().rearrange("(dt p) c -> p dt c", p=P)
    wg_r = wgx.ap().rearrange("p (ft dt f) -> p ft dt f", ft=FT, dt=DT)
    wu_r = wux.ap().rearrange("p (ft dt f) -> p ft dt f", ft=FT, dt=DT)
    wd_r = wdx.ap().rearrange(
        "p (db fb fi d) -> p db fb fi d", db=DB, fb=FT // FTB, fi=FTB
    )
    yT_ap = yT.ap()

    sizes = _group_plan(C)
    groups = []
    g0 = 0
    for gc in sizes:
        groups.append((g0, gc))
        g0 += gc
    gmax = max(sizes)

    with TileContext(nc) as tc:
        with (
            tc.tile_pool(name="warm", bufs=1) as warm_pool,
            tc.tile_pool(name="xg", bufs=2) as xg_pool,
            tc.tile_pool(name="xu", bufs=2) as xu_pool,
            tc.tile_pool(name="wgu", bufs=4) as wgu_pool,
            tc.tile_pool(name="ht", bufs=FT + 4) as ht_pool,
            tc.tile_pool(name="wdp", bufs=4) as wd_pool,
            tc.tile_pool(name="act", bufs=2) as act_pool,
            tc.tile_pool(name="out", bufs=4) as out_pool,
            tc.tile_pool(name="ps1", bufs=1, space="PSUM") as ps1_pool,
            tc.tile_pool(name="ps2", bufs=6, space="PSUM") as ps2_pool,
        ):
            # Warm-up: keep TensorE busy while the first tiles stream in, so
            # the HAM clock gate reaches 2.4 GHz before real matmuls start.
            wsrc = warm_pool.tile([P, 512], bf16, name="wsrc")
            nc.vector.memset(wsrc.bitcast(fp32)[:], 0.0)
            wps = ps1_pool.tile([P, 512], fp32, name="psg")
            NWARM = 40
            for wi in range(NWARM):
                nc.tensor.matmul(
                    wps[:, :256],
                    wsrc[:, :P],
                    wsrc[:, :256],
                    start=(wi == 0),
                    stop=(wi == NWARM - 1),
                )
            for gi, (g0, gc) in enumerate(groups):
                spans = _chunk_plan(gc)

                # hoist the first f-tile's weights ahead of the token slab so
                # stage 1 can start as soon as xg's first slice lands
                wgu_pre = {}
                for ft in range(1):
                    wgt = wgu_pool.tile([P, DT, P], bf16, name="wgt")
                    nc.sync.dma_start(out=wgt[:], in_=wg_r[:, ft, :, :])
                    wut = wgu_pool.tile([P, DT, P], bf16, name="wut")
                    nc.sync.dma_start(out=wut[:], in_=wu_r[:, ft, :, :])
                    wgu_pre[ft] = (wgt, wut)

                xg = xg_pool.tile([P, DT, gmax], bf16, name="xg")
                xu = xu_pool.tile([P, DT, gmax], bf16, name="xu")
                # per-dt loads (gate slab first) so stage-1 matmuls can start
                # on the first slice. For the first group, prime the two
                # slabs on separate queues so both land in half the time
                # (scalar is otherwise idle until the first silu).
                xu_eng = nc.scalar if gi == 0 else nc.sync
                for dt_i in range(DT):
                    nc.sync.dma_start(
                        out=xg[:, dt_i, :gc], in_=xT_r[:, dt_i, g0 : g0 + gc]
                    )
                for dt_i in range(DT):
                    xu_eng.dma_start(
                        out=xu[:, dt_i, :gc], in_=xuT_r[:, dt_i, g0 : g0 + gc]
                    )

                # ---- stage 1: hT[f, c] = silu(wg.T x) * (wu.T xs) ----
                ht_tiles = []
                for ft in range(FT):
                    if ft in wgu_pre:
                        wgt, wut = wgu_pre.pop(ft)
                    else:
                        wgt = wgu_pool.tile([P, DT, P], bf16, name="wgt")
                        nc.sync.dma_start(out=wgt[:], in_=wg_r[:, ft, :, :])
                        wut = wgu_pool.tile([P, DT, P], bf16, name="wut")
                        nc.sync.dma_start(out=wut[:], in_=wu_r[:, ft, :, :])
                    ht = ht_pool.tile([P, gmax], bf16, name="ht")
                    ht_tiles.append(ht)
                    for c0, cw in spans:
                        psg = ps1_pool.tile([P, 512], fp32, name="psg")
                        for dt_i in range(DT):
                            nc.tensor.matmul(
                                psg[:, :cw],
                                wgt[:, dt_i, :],
                                xg[:, dt_i, c0 : c0 + cw],
                                start=(dt_i == 0),
                                stop=(dt_i == DT - 1),
                            )
                        psu = ps1_pool.tile([P, 512], fp32, name="psu")
                        for dt_i in range(DT):
                            nc.tensor.matmul(
                                psu[:, :cw],
                                wut[:, dt_i, :],
                                xu[:, dt_i, c0 : c0 + cw],
                                start=(dt_i == 0),
                                stop=(dt_i == DT - 1),
                            )
                        sil = act_pool.tile([P, 512], fp32, name="sil")
                        nc.scalar.activation(sil[:, :cw], psg[:, :cw], silu_fn)
                        nc.vector.tensor_tensor(
                            out=ht[:, c0 : c0 + cw],
                            in0=sil[:, :cw],
                            in1=psu[:, :cw],
                            op=mult_op,
                        )

                # ---- stage 2 (transposed): yT[d, c] = sum_f wd[f, d] hT[f, c]
                # stationary = wd tile [128f, 128d], moving = hT span; wd
                # streams exactly once per group.
                for db in range(DB):
                    ps_out = [
                        ps2_pool.tile([P, 512], fp32, name="pso") for _ in spans
                    ]
                    for fb in range(FT // FTB):
                        wdt = wd_pool.tile([P, FTB, P], bf16, name="wdt")
                        nc.scalar.dma_start(
                            out=wdt[:], in_=wd_r[:, db, fb, :, :]
                        )
                        for fi in range(FTB):
                            ft = fb * FTB + fi
                            for si, (c0, cw) in enumerate(spans):
                                nc.tensor.matmul(
                                    ps_out[si][:, :cw],
                                    wdt[:, fi, :],
                                    ht_tiles[ft][:, c0 : c0 + cw],
                                    start=(ft == 0),
                                    stop=(ft == FT - 1),
                                )
                    for si, (c0, cw) in enumerate(spans):
                        ot = out_pool.tile([P, 512], fp32, name="ot")
                        if si % 2 == 0:
                            nc.vector.tensor_scalar_mul(
                                ot[:, :cw], ps_out[si][:, :cw], 1.0
                            )
                        else:
                            # spread evictions across engines so the bank
                            # ring frees faster at d-tile boundaries
                            nc.scalar.activation(
                                ot[:, :cw],
                                ps_out[si][:, :cw],
                                mybir.ActivationFunctionType.Copy,
                            )
                        # gpsimd/scalar queues: the sync queue must stay clear
                        # for the next group's token-slab prefetch (last
                        # group has no successor, so sync can help its tail)
                        if gi == len(groups) - 1:
                            dma_eng = (nc.gpsimd, nc.scalar, nc.sync)[si % 3]
                        else:
                            dma_eng = nc.gpsimd if si % 2 == 0 else nc.scalar
                        dma_eng.dma_start(
                            out=yT_ap[db * P : (db + 1) * P, g0 + c0 : g0 + c0 + cw],
                            in_=ot[:, :cw],
                        )
    nc.compile()
    return nc


def _get_program(C):
    if C not in _programs:
        _programs[C] = _build_program(C)
    return _programs[C]


def _route(xf, router_w):
    """Host router, float64 (all f32 evaluation orders agree on this input's
    top-2 sets; f64 is the stable reference ranking). Mirrors
    softmax -> top_k(2) -> renormalize from the reference."""
    logits = xf.astype(np.float64) @ router_w.astype(np.float64).T
    logits -= logits.max(axis=-1, keepdims=True)
    sm = np.exp(logits)
    sm /= sm.sum(axis=-1, keepdims=True)
    top = np.argsort(-sm, axis=-1, kind="stable")[:, :TOP_K]
    tsc = np.take_along_axis(sm, top, axis=1)
    tsc = tsc / tsc.sum(axis=-1, keepdims=True)
    return top, tsc


def _permute_wgu(w, bf):
    """[D, F] -> tile-contiguous [128p, (32ft 8dt 128f')]"""
    D, F = w.shape
    v = w.reshape(D // P, P, F // P, P).transpose(1, 2, 0, 3)
    return np.ascontiguousarray(v.astype(bf).reshape(P, -1))


def _permute_wd(w, bf):
    """[F, D] -> tile-contiguous [128p, (8db FT/FTB FTB 128d')]"""
    F, D = w.shape
    nfb = F // P // FTB
    v = w.reshape(nfb, FTB, P, D // P, P).transpose(2, 3, 0, 1, 4)
    return np.ascontiguousarray(v.astype(bf).reshape(P, -1))


def kernel(x, router_w, w_gate, w_up, w_down):
    global LAST_EXEC_NS
    import ml_dtypes
    from concourse.bass_utils import run_bass_kernel_spmd

    bf = ml_dtypes.bfloat16

    trace = os.environ.get("MOE_TRACE", "0") == "1"
    if trace:
        _ensure_axon_hooks()

    x = np.asarray(x, dtype=np.float32)
    router_w = np.asarray(router_w, dtype=np.float32)

    B, T, D = x.shape
    N = B * T
    xf = np.ascontiguousarray(x.reshape(N, D))

    top, tsc = _route(xf, router_w)

    tok_rows = []
    tok_wts = []
    for e in range(N_EXPERTS):
        mask = top == e
        rows = np.nonzero(mask.any(axis=1))[0]
        wts = tsc[mask].astype(np.float32)
        tok_rows.append(rows)
        tok_wts.append(wts)

    cmax = max(max(len(r) for r in tok_rows), 1)
    C = max(((cmax + P - 1) // P) * P, 256)

    nc = _get_program(C)

    in_maps = []
    for e in range(N_EXPERTS):
        rows = tok_rows[e]
        xg = np.zeros((C, D), np.float32)
        xg[: len(rows)] = xf[rows]
        xs = np.zeros((C, D), np.float32)
        xs[: len(rows)] = xf[rows] * tok_wts[e][:, None]
        in_maps.append(
            {
                "xT": np.ascontiguousarray(xg.T.astype(bf)),
                "xuT": np.ascontiguousarray(xs.T.astype(bf)),
                "wgx": _permute_wgu(np.asarray(w_gate[e], np.float32), bf),
                "wux": _permute_wgu(np.asarray(w_up[e], np.float32), bf),
                "wdx": _permute_wd(np.asarray(w_down[e], np.float32), bf),
            }
        )

    res = run_bass_kernel_spmd(nc, in_maps, list(range(N_EXPERTS)), trace=trace)
    if trace:
        LAST_EXEC_NS = res.exec_time_ns

    out = np.zeros((N, D), np.float32)
    for e in range(N_EXPERTS):
        rows = tok_rows[e]
        out[rows] += res.results[e]["yT"][:, : len(rows)].T
    return out.reshape(B, T, D)
